# revision 14
# baseline (speedup 1.0000x reference)
"""CrossBatchAttention Trainium2 kernel — 8-core tensor-parallel SPMD.

Layout strategy: every on-chip tensor is kept in transposed [feature, batch]
layout so the TensorEngine contraction dim is always on partitions and no
on-chip transposes are needed. Host numpy does all transposes / casts /
shard slicing, and adds the residual hidden_states at the end.

v2: all large matmuls run fp8 with MatmulPerfMode.DoubleRow (2 k-tiles per
instruction, ~1.5x PE throughput). Weights are pre-scaled by 64 into
fp8e4m3 range; X/V/P stream in fp8. Attention probabilities use fp8e5m2
(wide dynamic range: exp(s - 4ln2) never overflows and the tail never
flushes to zero); the 2^-4 prescale cancels in the softmax normalization.

Per core c (of 8):
  phase 1: QT/KT [512,2048] bf16 (64x scaled), V [2048,512] fp8 (4 local
           heads), g1X (gate W1 X-part) — weights resident, X^T streamed
           in batch-quarters, all matmuls fp8-DoubleRow.
  phase 2: per (head, batch-quarter): S^T = K^T@Q^T per j-tile (bf16),
           ACT Exp((SCALE/4096)*s + bias) -> fp8e5m2, diagonal zeroed,
           denominator via all-ones DoubleRow matmul, O^T = V@P^T
           (DoubleRow), normalize with reciprocal_approx_fast.
           AllGather O^T per head.
  phase 3: cross^T[hid-shard] = Wo[:, shard]^T @ OT_full (column-parallel,
           DoubleRow), k-grouped by AG chunk; last group i-chunk-major and
           feeds the gate chain per chunk.
  phase 4 (pipelined per i-chunk inside phase 3's last group):
           g1C partial (DoubleRow) -> ReduceScatter(gh) -> gelu(fp8) ->
           AllGather(g^T) -> logits (DoubleRow) -> sigmoid -> out^T.
Host: concat 8 [512,2048] shards, transpose, add X -> [2048,4096] f32.
"""

import numpy as np
import ml_dtypes

import concourse.bass as bass
import concourse.mybir as mybir
import concourse.tile as tile
from concourse import bacc
from concourse import bass_utils

BF16 = mybir.dt.bfloat16
F32 = mybir.dt.float32
F8 = mybir.dt.float8e4
F8E5 = mybir.dt.float8e5
DR = mybir.MatmulPerfMode.DoubleRow
W_SCALE = 64.0

B = 2048
HID = 4096
NH = 32
HD = 128
GH = 1024
NC_ = 8
HPC = NH // NC_          # heads per core = 4
HS = HID // NC_          # hid shard = 512
GS = GH // NC_           # gate-hidden shard = 128
SCALE = 1.0 / float(np.sqrt(HD))
EXP_SHIFT = 4 * float(np.log(2.0))   # exp(s - 4ln2): cancels in softmax

KT_TILES = HID // 128    # 32 k-tiles over the 4096 contraction
JT = B // 128            # 16 j-tiles over keys
IC = B // 512            # 4 i-chunks of 512 over batch

GELU_FUNC = mybir.ActivationFunctionType.Gelu


def _build_program():
    nc = bacc.Bacc(
        "TRN2",
        target_bir_lowering=False,
        debug=False,
        enable_asserts=False,
        num_devices=NC_,
    )

    # ---- I/O declarations (per-core shapes) ----
    xt_f8 = nc.dram_tensor("xt_f8", [HID, B], F8, kind="ExternalInput").ap()
    wq_d = nc.dram_tensor("wq", [HID, HS], F8, kind="ExternalInput").ap()
    wk_d = nc.dram_tensor("wk", [HID, HS], F8, kind="ExternalInput").ap()
    wv_d = nc.dram_tensor("wv", [HID, HS], F8, kind="ExternalInput").ap()
    wo_d = nc.dram_tensor("wo", [HID, HS], F8, kind="ExternalInput").ap()
    gw1x_d = nc.dram_tensor("gw1x", [HID, GS], F8, kind="ExternalInput").ap()
    gw1c_d = nc.dram_tensor("gw1c", [HS, GH], F8, kind="ExternalInput").ap()
    gw2_d = nc.dram_tensor("gw2", [GH, HS], F8, kind="ExternalInput").ap()
    gb1_d = nc.dram_tensor("gb1", [GS, 1], F32, kind="ExternalInput").ap()
    gb2_d = nc.dram_tensor("gb2", [128, 4], F32, kind="ExternalInput").ap()
    maskb_d = nc.dram_tensor("maskb", [128, JT], F32, kind="ExternalInput").ap()
    diagm_d = nc.dram_tensor("diagm", [128, 128], F8E5, kind="ExternalInput").ap()
    out_d = nc.dram_tensor("out", [HS, B], BF16, kind="ExternalOutput").ap()

    groups = [list(range(NC_))]

    with tile.TileContext(nc) as tc:
        with (
            tc.tile_pool(name="persist", bufs=1) as persist,
            tc.tile_pool(name="psum", bufs=1, space="PSUM") as psum,
            tc.tile_pool(name="dram", bufs=1, space="DRAM") as dram,
        ):
            # ---------- persistent SBUF ----------
            qt_sb = persist.tile([128, HPC, B], BF16)     # [d, head, i] 64x
            kt_sb = persist.tile([128, HPC, B], BF16)     # 64x scaled
            v_sb = persist.tile([128, JT, HS], F8)        # [j_in, j_tile, hd]
            g1x_sb = persist.tile([128, B], F32)          # gate W1 X-part
            maskb_sb = persist.tile([128, JT], F32)
            diagm_sb = persist.tile([128, 128], F8E5)
            ones2_sb = persist.tile([128, 2, 128], F8)
            gb1_sb = persist.tile([GS, 1], F32)
            gb2_sb = persist.tile([128, 4], F32)

            nc.vector.memset(ones2_sb, 1.0)

            # ---------- DRAM bounce buffers for collectives ----------
            # separate dram tiles per (head, half): avoids false
            # whole-tile write-after-read deps against in-flight AGs
            ag_in = [[None, None] for _ in range(HPC)]
            ag_out = [[None, None] for _ in range(HPC)]
            for h in range(HPC):
                for hf in range(2):
                    ag_in[h][hf] = dram.tile(
                        [128, B // 2], F8, name=f"ag_in{h}_{hf}"
                    )
                    ag_out[h][hf] = dram.tile(
                        [NC_ * 128, B // 2], F8, addr_space="Shared",
                        name=f"ag_out{h}_{hf}"
                    )
            # gate chain per i-chunk; tail chunks 2,3 use fp8 RS payloads
            rs_in_c, rs_out_c, ag2_in_c, ag2_out_c = {}, {}, {}, {}
            for icc in range(IC):
                rs_in_c[icc] = dram.tile([GH, 512], F8, name=f"rs_in{icc}")
                rs_out_c[icc] = dram.tile([GS, 512], F8, name=f"rs_out{icc}")
                ag2_in_c[icc] = dram.tile([GS, 512], F8, name=f"ag2_in{icc}")
                ag2_out_c[icc] = dram.tile([GH, 512], F8, addr_space="Shared",
                                           name=f"ag2_out{icc}")
            # i-chunks 0,1 share one paired AG2 (saves a CC op mid-chain)
            ag2_in_p = dram.tile([GS, 1024], F8, name="ag2_in_p")
            ag2_out_p = dram.tile([GH, 1024], F8, addr_space="Shared",
                                  name="ag2_out_p")
            warm_rs_i = dram.tile([GH, 64], BF16)
            warm_rs_o = dram.tile([GS, 64], BF16)
            warm_ag_i = dram.tile([GS, 64], BF16)
            warm_ag_o = dram.tile([GH, 64], BF16, addr_space="Shared")
            nc.gpsimd.collective_compute(
                "ReduceScatter", mybir.AluOpType.add, replica_groups=groups,
                ins=[warm_rs_i.opt()], outs=[warm_rs_o.opt()],
            )
            nc.gpsimd.collective_compute(
                "AllGather", mybir.AluOpType.bypass, replica_groups=groups,
                ins=[warm_ag_i.opt()], outs=[warm_ag_o.opt()],
            )

            # =====================================================
            # Phase 1: projections, weights resident, X streamed
            # =====================================================
            with tc.tile_pool(name="p1", bufs=1) as p1:
                wq_sb = p1.tile([128, KT_TILES, HS], F8, tag="wq", bufs=1)
                wk_sb = p1.tile([128, KT_TILES, HS], F8, tag="wk", bufs=1)
                wv_sb = p1.tile([128, KT_TILES, HS], F8, tag="wv", bufs=1)
                gw1x_sb = p1.tile([128, KT_TILES, GS], F8, tag="gw1x", bufs=1)

                def load_w(dst, src, ncols):
                    for hh in range(4):
                        nc.sync.dma_start(
                            out=dst[:, hh * 8:(hh + 1) * 8, :],
                            in_=src[hh * 1024:(hh + 1) * 1024, :].rearrange(
                                "(t p) m -> p t m", p=128
                            ),
                        )

                load_w(wq_sb, wq_d, HS)
                xt_tiles = []
                for q in range(IC):
                    xt_q = p1.tile([128, KT_TILES, 512], F8, tag="xt", bufs=2)
                    xt_tiles.append(xt_q)
                isl0 = slice(0, 512)
                for kk in range(4):
                    nc.sync.dma_start(
                        out=xt_tiles[0][:, kk * 8:(kk + 1) * 8, :],
                        in_=xt_f8[kk * 1024:(kk + 1) * 1024, isl0].rearrange(
                            "(t p) i -> p t i", p=128
                        ),
                    )
                load_w(wk_sb, wk_d, HS)
                load_w(wv_sb, wv_d, HS)
                load_w(gw1x_sb, gw1x_d, GS)
                nc.sync.dma_start(out=maskb_sb, in_=maskb_d)
                nc.sync.dma_start(out=diagm_sb, in_=diagm_d)
                nc.sync.dma_start(out=gb1_sb, in_=gb1_d)
                nc.sync.dma_start(out=gb2_sb, in_=gb2_d)

                for q in range(IC):  # 4 quarters of 512 batch elems
                    isl = slice(q * 512, (q + 1) * 512)
                    xt_q = xt_tiles[q]
                    if q + 1 < IC:
                        nxt = slice((q + 1) * 512, (q + 2) * 512)
                        for kk in range(4):
                            nc.sync.dma_start(
                                out=xt_tiles[q + 1][:, kk * 8:(kk + 1) * 8, :],
                                in_=xt_f8[kk * 1024:(kk + 1) * 1024,
                                          nxt].rearrange(
                                    "(t p) i -> p t i", p=128
                                ),
                            )

                    for wsb, dst in ((wq_sb, qt_sb), (wk_sb, kt_sb)):
                        for m in range(4):
                            ps = psum.tile([128, 512], F32, tag="mm", bufs=3,
                                           name="ps_pr")
                            for k in range(KT_TILES // 2):
                                nc.tensor.matmul(
                                    ps,
                                    lhsT=wsb[:, 2 * k:2 * k + 2,
                                             m * 128:(m + 1) * 128],
                                    rhs=xt_q[:, 2 * k:2 * k + 2, :],
                                    start=(k == 0),
                                    stop=(k == KT_TILES // 2 - 1),
                                    perf_mode=DR,
                                )
                            nc.vector.tensor_copy(dst[:, m, isl], ps)
                    # V in natural [j, d] layout: lhsT = X^T tiles
                    for it in range(4):  # 4 i-tiles of 128 in this quarter
                        ps = psum.tile([128, 512], F32, tag="mm", bufs=3,
                                       name="ps_v")
                        for k in range(KT_TILES // 2):
                            nc.tensor.matmul(
                                ps,
                                lhsT=xt_q[:, 2 * k:2 * k + 2,
                                          it * 128:(it + 1) * 128],
                                rhs=wv_sb[:, 2 * k:2 * k + 2, :],
                                start=(k == 0),
                                stop=(k == KT_TILES // 2 - 1),
                                perf_mode=DR,
                            )
                        nc.vector.tensor_scalar_mul(
                            v_sb[:, q * 4 + it, :], ps, 1.0 / W_SCALE
                        )
                    # gate W1 X-part (gh-shard output)
                    ps = psum.tile([128, 512], F32, tag="mm", bufs=3,
                                   name="ps_g1x")
                    for k in range(KT_TILES // 2):
                        nc.tensor.matmul(
                            ps,
                            lhsT=gw1x_sb[:, 2 * k:2 * k + 2, :],
                            rhs=xt_q[:, 2 * k:2 * k + 2, :],
                            start=(k == 0),
                            stop=(k == KT_TILES // 2 - 1),
                            perf_mode=DR,
                        )
                    nc.vector.tensor_scalar_mul(
                        g1x_sb[:, isl], ps, 1.0 / W_SCALE
                    )

            # =====================================================
            # Phases 2-4 merged: attention blocks, out_proj chunks and
            # the gate-MLP chain interleaved so collectives overlap PE
            # work. Second-half head order h3..h0 matches the order in
            # which out_proj consumes the hf=1 AllGathers.
            # =====================================================
            with (
                tc.tile_pool(name="p2", bufs=1) as p2,
                tc.tile_pool(name="p34", bufs=1) as p34,
            ):
                # phase-3/4 weights load up front, hidden under phase 2
                cacc = p34.tile([128, 4, B], BF16, tag="cacc", bufs=1)
                wo_sb = p34.tile([128, KT_TILES, HS], F8, tag="wo", bufs=1)
                nc.sync.dma_start(
                    out=wo_sb, in_=wo_d.rearrange("(t p) m -> p t m", p=128)
                )
                gw1c_sb = p34.tile([128, 4, GH], F8, tag="gw1c", bufs=1)
                nc.sync.dma_start(
                    out=gw1c_sb, in_=gw1c_d.rearrange("(t p) m -> p t m", p=128)
                )
                gw2_sb = p34.tile([128, NC_, HS], F8, tag="gw2", bufs=1)
                nc.sync.dma_start(
                    out=gw2_sb, in_=gw2_d.rearrange("(t p) m -> p t m", p=128)
                )
                g1c_sb = p34.tile([128, B], BF16, tag="g1c", bufs=1)
                g1c_sb8 = p34.tile([128, B], F8, tag="g1c8", bufs=1)
                gtf_tiles = {}

                def attn_block(h, q):
                    qsl = slice(q * 512, (q + 1) * 512)
                    den_ps = psum.tile([128, 512], F32, tag="den", bufs=2)
                    ot_ps = psum.tile([128, 512], F32, tag="ot", bufs=2)
                    pt = p2.tile([128, JT, 512], F8E5, tag="pt", bufs=2)
                    for j in range(JT):
                        st = psum.tile([128, 512], F32, tag="mm", bufs=3,
                                       name="st")
                        nc.tensor.matmul(
                            st,
                            lhsT=kt_sb[:, h, j * 128:(j + 1) * 128],
                            rhs=qt_sb[:, h, qsl],
                            start=True,
                            stop=True,
                        )
                        # qt/kt are 64x: fold 1/4096 into the exp scale
                        nc.scalar.activation(
                            pt[:, j, :],
                            st,
                            mybir.ActivationFunctionType.Exp,
                            bias=maskb_sb[:, j:j + 1],
                            scale=SCALE / (W_SCALE * W_SCALE),
                        )
                        # zero the self-attention diagonal block
                        if j // 4 == q:
                            c0 = (j % 4) * 128
                            nc.vector.tensor_mul(
                                pt[:, j, c0:c0 + 128],
                                pt[:, j, c0:c0 + 128],
                                diagm_sb,
                            )
                    for jj in range(JT // 2):
                        nc.tensor.matmul(
                            den_ps,
                            lhsT=ones2_sb,
                            rhs=pt[:, 2 * jj:2 * jj + 2, :],
                            start=(jj == 0),
                            stop=(jj == JT // 2 - 1),
                            perf_mode=DR,
                        )
                        nc.tensor.matmul(
                            ot_ps,
                            lhsT=v_sb[:, 2 * jj:2 * jj + 2,
                                      h * 128:(h + 1) * 128],
                            rhs=pt[:, 2 * jj:2 * jj + 2, :],
                            start=(jj == 0),
                            stop=(jj == JT // 2 - 1),
                            perf_mode=DR,
                        )
                    rec = p2.tile([128, 512], F32, tag="rec", bufs=2)
                    nc.vector.reciprocal_approx_fast(out=rec, in_=den_ps)
                    otc = p2.tile([128, 512], F8, tag="otc", bufs=2)
                    nc.vector.tensor_mul(otc, ot_ps, rec)
                    nc.sync.dma_start(
                        out=ag_in[h][q // 2][:, (q % 2) * 512:
                                             (q % 2) * 512 + 512],
                        in_=otc,
                    )

                def ag_ot(h, hf):
                    nc.gpsimd.collective_compute(
                        "AllGather",
                        mybir.AluOpType.bypass,
                        replica_groups=groups,
                        ins=[ag_in[h][hf].opt()],
                        outs=[ag_out[h][hf].opt()],
                    )

                otg_tiles = {}

                def load_otg(t, ic):
                    otg = p34.tile([128, NC_, 512], F8, tag="otg", bufs=6,
                                   name="otg")
                    nc.sync.dma_start(
                        out=otg,
                        in_=ag_out[t][ic // 2][:, (ic % 2) * 512:
                                               (ic % 2) * 512 + 512].rearrange(
                            "(r p) i -> p r i", p=128
                        ),
                    )
                    otg_tiles[(t, ic)] = otg

                def outproj_chunk(ic, t_list, first, last):
                    """One psum accumulation group per m over all t in
                    t_list (16 DR when t_list is all 4 slots). ACT does
                    the psum->cacc copy; DVE only accumulates the late
                    t0 slot of the tail chunks."""
                    csl = slice(ic * 512, (ic + 1) * 512)
                    for m in range(4):
                        ps = psum.tile([128, 512], F32, tag="mm", bufs=3,
                                       name="ps_wo")
                        for ti, t in enumerate(t_list):
                            otg = otg_tiles[(t, ic)]
                            for r in range(NC_ // 2):
                                nc.tensor.matmul(
                                    ps,
                                    lhsT=wo_sb[:, t * NC_ + 2 * r:
                                               t * NC_ + 2 * r + 2,
                                               m * 128:(m + 1) * 128],
                                    rhs=otg[:, 2 * r:2 * r + 2, :],
                                    start=(ti == 0 and r == 0),
                                    stop=(ti == len(t_list) - 1
                                          and r == NC_ // 2 - 1),
                                    perf_mode=DR,
                                )
                        if first:
                            nc.vector.tensor_scalar_mul(
                                cacc[:, m, csl], ps, 1.0 / W_SCALE
                            )
                        else:
                            nc.vector.scalar_tensor_tensor(
                                cacc[:, m, csl], ps, 1.0 / W_SCALE,
                                cacc[:, m, csl],
                                op0=mybir.AluOpType.mult,
                                op1=mybir.AluOpType.add,
                            )

                def gate_g1c(ic):
                    csl = slice(ic * 512, (ic + 1) * 512)
                    # fp8 copy of this cross^T chunk for the DR g1C matmul
                    cf8 = p34.tile([128, 4, 512], F8, tag="cf8", bufs=2)
                    for m in range(4):
                        nc.vector.tensor_copy(cf8[:, m, :], cacc[:, m, csl])
                    rdt = F8
                    for gm in range(NC_):  # 8 gh-tiles of g1C partial
                        ps = psum.tile([128, 512], F32, tag="mm", bufs=3,
                                       name="ps_g1c")
                        for r in range(2):
                            nc.tensor.matmul(
                                ps,
                                lhsT=gw1c_sb[:, 2 * r:2 * r + 2,
                                             gm * 128:(gm + 1) * 128],
                                rhs=cf8[:, 2 * r:2 * r + 2, :],
                                start=(r == 0),
                                stop=(r == 1),
                                perf_mode=DR,
                            )
                        g1c_ch = p34.tile([128, 512], rdt, tag="g1cch",
                                          bufs=4, name=f"g1cch{ic % 2}")
                        nc.vector.tensor_scalar_mul(g1c_ch, ps, 1.0 / W_SCALE)
                        nc.sync.dma_start(
                            out=rs_in_c[ic][gm * 128:(gm + 1) * 128, :],
                            in_=g1c_ch,
                        )

                def rs_issue(ic):
                    nc.gpsimd.collective_compute(
                        "ReduceScatter",
                        mybir.AluOpType.add,
                        replica_groups=groups,
                        ins=[rs_in_c[ic].opt()],
                        outs=[rs_out_c[ic].opt()],
                    )

                def pass_b(ic):
                    csl = slice(ic * 512, (ic + 1) * 512)
                    g1c_t = g1c_sb8
                    nc.gpsimd.dma_start(out=g1c_t[:, csl], in_=rs_out_c[ic])
                    gsum = p34.tile([128, 512], F32, tag="gsum", bufs=2)
                    nc.gpsimd.tensor_add(gsum, g1x_sb[:, csl], g1c_t[:, csl])
                    gt_ch = p34.tile([128, 512], F8, tag="gt", bufs=2)
                    nc.scalar.activation(gt_ch, gsum, GELU_FUNC,
                                         bias=gb1_sb, scale=1.0)
                    if ic < 2:
                        nc.gpsimd.dma_start(
                            out=ag2_in_p[:, ic * 512:(ic + 1) * 512],
                            in_=gt_ch)
                        if ic == 0:
                            return
                        i_, o_ = ag2_in_p, ag2_out_p
                    else:
                        nc.gpsimd.dma_start(out=ag2_in_c[ic], in_=gt_ch)
                        i_, o_ = ag2_in_c[ic], ag2_out_c[ic]
                    nc.gpsimd.collective_compute(
                        "AllGather",
                        mybir.AluOpType.bypass,
                        replica_groups=groups,
                        ins=[i_.opt()],
                        outs=[o_.opt()],
                    )

                def load_gtf(ic):
                    gtf = p34.tile([128, NC_, 512], F8, tag="gtf", bufs=4,
                                   name=f"gtf{ic}")
                    if ic < 2:
                        src_ = ag2_out_p[:, ic * 512:(ic + 1) * 512]
                    else:
                        src_ = ag2_out_c[ic]
                    nc.scalar.dma_start(
                        out=gtf,
                        in_=src_.rearrange("(r p) i -> p r i", p=128),
                    )
                    gtf_tiles[ic] = gtf

                def gw2_chunk(ic):
                    csl = slice(ic * 512, (ic + 1) * 512)
                    gtf = gtf_tiles[ic]
                    for m in range(4):
                        ps = psum.tile([128, 512], F32, tag="mm", bufs=3,
                                       name="ps_gw2")
                        for r in range(NC_ // 2):
                            nc.tensor.matmul(
                                ps,
                                lhsT=gw2_sb[:, 2 * r:2 * r + 2,
                                            m * 128:(m + 1) * 128],
                                rhs=gtf[:, 2 * r:2 * r + 2, :],
                                start=(r == 0),
                                stop=(r == NC_ // 2 - 1),
                                perf_mode=DR,
                            )
                        gate_ch = p34.tile([128, 512], BF16, tag="gate",
                                           bufs=2)
                        nc.scalar.activation(
                            gate_ch, ps,
                            mybir.ActivationFunctionType.Sigmoid,
                            bias=gb2_sb[:, m:m + 1], scale=1.0 / W_SCALE,
                        )
                        outt = p34.tile([128, 512], BF16, tag="outt",
                                        bufs=2)
                        nc.vector.tensor_mul(outt, gate_ch, cacc[:, m, csl])
                        nc.sync.dma_start(
                            out=out_d[m * 128:(m + 1) * 128, csl], in_=outt
                        )

                # ---- schedule ----
                # first half: quarters 0,1 head order 3..0 so the hf0
                # AllGathers land in the order out_proj consumes them
                for h in (3, 2, 1, 0):
                    attn_block(h, 0)
                    attn_block(h, 1)
                    ag_ot(h, 0)
                # second half (h3..h0) interleaved with out_proj + gate
                attn_block(3, 2)
                attn_block(3, 3)
                ag_ot(3, 1)
                for t in (3, 2, 1, 0):
                    load_otg(t, 0)
                outproj_chunk(0, (3, 2, 1, 0), first=True, last=True)
                gate_g1c(0)
                rs_issue(0)
                attn_block(2, 2)
                attn_block(2, 3)
                ag_ot(2, 1)
                for t in (3, 2, 1, 0):
                    load_otg(t, 1)
                outproj_chunk(1, (3, 2, 1, 0), first=True, last=True)
                gate_g1c(1)
                rs_issue(1)
                attn_block(1, 2)
                attn_block(1, 3)
                ag_ot(1, 1)
                attn_block(0, 2)
                attn_block(0, 3)
                ag_ot(0, 1)
                pass_b(0)
                pass_b(1)
                load_gtf(0)
                load_gtf(1)
                for t in (3, 2, 1):
                    load_otg(t, 2)
                    load_otg(t, 3)
                outproj_chunk(2, (3, 2, 1), first=True, last=False)
                outproj_chunk(3, (3, 2, 1), first=True, last=False)
                load_otg(0, 2)
                load_otg(0, 3)
                outproj_chunk(2, (0,), first=False, last=True)
                gate_g1c(2)
                rs_issue(2)
                outproj_chunk(3, (0,), first=False, last=True)
                gate_g1c(3)
                rs_issue(3)
                gw2_chunk(0)
                gw2_chunk(1)
                pass_b(2)
                load_gtf(2)
                pass_b(3)
                load_gtf(3)
                gw2_chunk(2)
                gw2_chunk(3)

    nc.compile()
    return nc


def _make_in_maps(inputs):
    f32 = np.float32
    f8 = ml_dtypes.float8_e4m3
    f8e5 = ml_dtypes.float8_e5m2
    X = np.asarray(inputs["hidden_states"], dtype=f32)
    mask = np.asarray(inputs["attention_mask"])
    Wq = np.asarray(inputs["Wq"], dtype=f32)
    Wk = np.asarray(inputs["Wk"], dtype=f32)
    Wv = np.asarray(inputs["Wv"], dtype=f32)
    Wo = np.asarray(inputs["Wo"], dtype=f32)
    gW1 = np.asarray(inputs["gW1"], dtype=f32)
    gb1 = np.asarray(inputs["gb1"], dtype=f32)
    gW2 = np.asarray(inputs["gW2"], dtype=f32)
    gb2 = np.asarray(inputs["gb2"], dtype=f32)

    XT = np.ascontiguousarray(X.T)                       # [4096, 2048]
    XT_f8 = XT.astype(f8)
    # Wo row permutation to match per-head AllGather chunk assembly:
    # OT_full row (t*1024 + r*128 + d) holds global head (4r+t), dim d.
    perm = np.empty(HID, dtype=np.int64)
    for t in range(HPC):
        for r in range(NC_):
            g = 4 * r + t
            perm[t * 1024 + r * 128:t * 1024 + (r + 1) * 128] = np.arange(
                g * 128, (g + 1) * 128
            )
    Wo_p = Wo[perm]
    # bias: -EXP_SHIFT for valid keys (prescales exp by 2^-4, cancels in
    # normalization), -1e30 for masked keys
    maskb = np.where(mask, -EXP_SHIFT, -1e30).astype(f32)    # [2048]
    maskb_t = np.ascontiguousarray(maskb.reshape(JT, 128).T)  # [128, 16]
    diagm = (1.0 - np.eye(128, dtype=f32)).astype(f8e5)

    in_maps = []
    for c in range(NC_):
        hsl = slice(c * HS, (c + 1) * HS)
        gsl = slice(c * GS, (c + 1) * GS)
        in_maps.append({
            "xt_f8": XT_f8,
            "wq": np.ascontiguousarray((Wq[:, hsl] * W_SCALE).astype(f8)),
            "wk": np.ascontiguousarray((Wk[:, hsl] * W_SCALE).astype(f8)),
            "wv": np.ascontiguousarray((Wv[:, hsl] * W_SCALE).astype(f8)),
            "wo": np.ascontiguousarray((Wo_p[:, hsl] * W_SCALE).astype(f8)),
            "gw1x": np.ascontiguousarray(
                (gW1[:HID, gsl] * W_SCALE).astype(f8)),
            "gw1c": np.ascontiguousarray(
                (gW1[HID + c * HS:HID + (c + 1) * HS] * W_SCALE).astype(f8)),
            "gw2": np.ascontiguousarray((gW2[:, hsl] * W_SCALE).astype(f8)),
            "gb1": np.ascontiguousarray(gb1[gsl].reshape(GS, 1)),
            "gb2": np.ascontiguousarray(gb2[hsl].reshape(4, 128).T),
            "maskb": maskb_t,
            "diagm": diagm,
        })
    return in_maps


_NC_CACHE = None


def _run(inputs, trace=False):
    global _NC_CACHE
    if _NC_CACHE is None:
        _NC_CACHE = _build_program()
    nc = _NC_CACHE
    in_maps = _make_in_maps(inputs)
    res = bass_utils.run_bass_kernel_spmd(
        nc, in_maps, core_ids=list(range(NC_)), trace=trace
    )
    shards = [np.asarray(res.results[c]["out"]).astype(np.float32)
              for c in range(NC_)]
    gated = np.concatenate(shards, axis=0).T  # gate * cross, [2048, 4096]
    out = np.asarray(inputs["hidden_states"], dtype=np.float32) + gated
    return np.ascontiguousarray(out), res


def kernel(**inputs) -> np.ndarray:
    out, _ = _run(inputs, trace=False)
    return out


# revision 16
# speedup vs baseline: 1.0493x; 1.0493x over previous
"""CrossBatchAttention Trainium2 kernel — 8-core tensor-parallel SPMD.

Layout strategy: every on-chip tensor is kept in transposed [feature, batch]
layout so the TensorEngine contraction dim is always on partitions and no
on-chip transposes are needed. Host numpy does all transposes / casts /
shard slicing, and adds the residual hidden_states at the end.

v2: all large matmuls run fp8 with MatmulPerfMode.DoubleRow (2 k-tiles per
instruction, ~1.5x PE throughput). Weights are pre-scaled by 64 into
fp8e4m3 range; X/V/P stream in fp8. Attention probabilities use fp8e5m2
(wide dynamic range: exp(s - 4ln2) never overflows and the tail never
flushes to zero); the 2^-4 prescale cancels in the softmax normalization.

Per core c (of 8):
  phase 1: QT/KT [512,2048] bf16 (64x scaled), V [2048,512] fp8 (4 local
           heads), g1X (gate W1 X-part) — weights resident, X^T streamed
           in batch-quarters, all matmuls fp8-DoubleRow.
  phase 2: per (head, batch-quarter): S^T = K^T@Q^T per j-tile (bf16),
           ACT Exp((SCALE/4096)*s + bias) -> fp8e5m2, diagonal zeroed,
           denominator via all-ones DoubleRow matmul, O^T = V@P^T
           (DoubleRow), normalize with reciprocal_approx_fast.
           AllGather O^T per head.
  phase 3: cross^T[hid-shard] = Wo[:, shard]^T @ OT_full (column-parallel,
           DoubleRow), k-grouped by AG chunk; last group i-chunk-major and
           feeds the gate chain per chunk.
  phase 4 (pipelined per i-chunk inside phase 3's last group):
           g1C partial (DoubleRow) -> ReduceScatter(gh) -> gelu(fp8) ->
           AllGather(g^T) -> logits (DoubleRow) -> sigmoid -> out^T.
Host: concat 8 [512,2048] shards, transpose, add X -> [2048,4096] f32.
"""

import numpy as np
import ml_dtypes

import concourse.bass as bass
import concourse.mybir as mybir
import concourse.tile as tile
from concourse import bacc
from concourse import bass_utils

BF16 = mybir.dt.bfloat16
F32 = mybir.dt.float32
F8 = mybir.dt.float8e4
F8E5 = mybir.dt.float8e5
DR = mybir.MatmulPerfMode.DoubleRow
W_SCALE = 64.0

B = 2048
HID = 4096
NH = 32
HD = 128
GH = 1024
NC_ = 8
HPC = NH // NC_          # heads per core = 4
HS = HID // NC_          # hid shard = 512
GS = GH // NC_           # gate-hidden shard = 128
SCALE = 1.0 / float(np.sqrt(HD))
EXP_SHIFT = 4 * float(np.log(2.0))   # exp(s - 4ln2): cancels in softmax

KT_TILES = HID // 128    # 32 k-tiles over the 4096 contraction
JT = B // 128            # 16 j-tiles over keys
IC = B // 512            # 4 i-chunks of 512 over batch

GELU_FUNC = mybir.ActivationFunctionType.Gelu


def _build_program():
    nc = bacc.Bacc(
        "TRN2",
        target_bir_lowering=False,
        debug=False,
        enable_asserts=False,
        num_devices=NC_,
    )

    # ---- I/O declarations (per-core shapes) ----
    xt_f8 = nc.dram_tensor("xt_f8", [HID, B], F8, kind="ExternalInput").ap()
    wq_d = nc.dram_tensor("wq", [HID, HS], F8, kind="ExternalInput").ap()
    wk_d = nc.dram_tensor("wk", [HID, HS], F8, kind="ExternalInput").ap()
    wv_d = nc.dram_tensor("wv", [HID, HS], F8, kind="ExternalInput").ap()
    wo_d = nc.dram_tensor("wo", [HID, HS], F8, kind="ExternalInput").ap()
    gw1x_d = nc.dram_tensor("gw1x", [HID, GS], F8, kind="ExternalInput").ap()
    gw1c_d = nc.dram_tensor("gw1c", [HS, GH], F8, kind="ExternalInput").ap()
    gw2_d = nc.dram_tensor("gw2", [GH, HS], F8, kind="ExternalInput").ap()
    gb1_d = nc.dram_tensor("gb1", [GS, 1], F32, kind="ExternalInput").ap()
    gb2_d = nc.dram_tensor("gb2", [128, 4], F32, kind="ExternalInput").ap()
    maskb_d = nc.dram_tensor("maskb", [128, JT], F32, kind="ExternalInput").ap()
    diagm_d = nc.dram_tensor("diagm", [128, 128], F8E5, kind="ExternalInput").ap()
    out_d = nc.dram_tensor("out", [HS, B], BF16, kind="ExternalOutput").ap()

    groups = [list(range(NC_))]

    with tile.TileContext(nc) as tc:
        with (
            tc.tile_pool(name="persist", bufs=1) as persist,
            tc.tile_pool(name="psum", bufs=1, space="PSUM") as psum,
            tc.tile_pool(name="dram", bufs=1, space="DRAM") as dram,
        ):
            # ---------- persistent SBUF ----------
            qt_sb = persist.tile([128, HPC, B], BF16)     # [d, head, i] 64x
            kt_sb = persist.tile([128, HPC, B], BF16)     # 64x scaled
            v_sb = persist.tile([128, JT, HS], F8)        # [j_in, j_tile, hd]
            g1x_sb = persist.tile([128, B], F32)          # gate W1 X-part
            maskb_sb = persist.tile([128, JT], F32)
            diagm_sb = persist.tile([128, 128], F8E5)
            ones2_sb = persist.tile([128, 2, 128], F8)
            gb1_sb = persist.tile([GS, 1], F32)
            gb2_sb = persist.tile([128, 4], F32)

            nc.vector.memset(ones2_sb, 1.0)

            # ---------- DRAM bounce buffers for collectives ----------
            # separate dram tiles per (head, half): avoids false
            # whole-tile write-after-read deps against in-flight AGs
            ag_in = [[None, None] for _ in range(HPC)]
            ag_out = [[None, None] for _ in range(HPC)]
            for h in range(HPC):
                for hf in range(2):
                    ag_in[h][hf] = dram.tile(
                        [128, B // 2], F8, name=f"ag_in{h}_{hf}"
                    )
                    ag_out[h][hf] = dram.tile(
                        [NC_ * 128, B // 2], F8, addr_space="Shared",
                        name=f"ag_out{h}_{hf}"
                    )
            # gate chain per i-chunk; tail chunks 2,3 use fp8 RS payloads
            rs_in_c, rs_out_c, ag2_in_c, ag2_out_c = {}, {}, {}, {}
            for icc in range(IC):
                rs_in_c[icc] = dram.tile([GH, 512], F8, name=f"rs_in{icc}")
                rs_out_c[icc] = dram.tile([GS, 512], F8, name=f"rs_out{icc}")
                ag2_in_c[icc] = dram.tile([GS, 512], F8, name=f"ag2_in{icc}")
                ag2_out_c[icc] = dram.tile([GH, 512], F8, addr_space="Shared",
                                           name=f"ag2_out{icc}")
            # i-chunks 0,1 share one paired AG2 (saves a CC op mid-chain)
            ag2_in_p = dram.tile([GS, 1024], F8, name="ag2_in_p")
            ag2_out_p = dram.tile([GH, 1024], F8, addr_space="Shared",
                                  name="ag2_out_p")
            # tail chunks 2,3 share one RS and one AG2
            rs_in_t = dram.tile([GH, 1024], F8, name="rs_in_t")
            rs_out_t = dram.tile([GS, 1024], F8, name="rs_out_t")
            ag2_in_t = dram.tile([GS, 1024], F8, name="ag2_in_t")
            ag2_out_t = dram.tile([GH, 1024], F8, addr_space="Shared",
                                  name="ag2_out_t")
            # late warmup: absorbs the CC wake-up penalty right before
            # the first real AllGather
            warm3_i = dram.tile([128, 64], F8)
            warm3_o = dram.tile([NC_ * 128, 64], F8, addr_space="Shared")
            warm_rs_i = dram.tile([GH, 64], BF16)
            warm_rs_o = dram.tile([GS, 64], BF16)
            warm_ag_i = dram.tile([GS, 64], BF16)
            warm_ag_o = dram.tile([GH, 64], BF16, addr_space="Shared")
            nc.gpsimd.collective_compute(
                "ReduceScatter", mybir.AluOpType.add, replica_groups=groups,
                ins=[warm_rs_i.opt()], outs=[warm_rs_o.opt()],
            )
            nc.gpsimd.collective_compute(
                "AllGather", mybir.AluOpType.bypass, replica_groups=groups,
                ins=[warm_ag_i.opt()], outs=[warm_ag_o.opt()],
            )

            # =====================================================
            # Phase 1: projections, weights resident, X streamed
            # =====================================================
            with tc.tile_pool(name="p1", bufs=1) as p1:
                wq_sb = p1.tile([128, KT_TILES, HS], F8, tag="wq", bufs=1)
                wk_sb = p1.tile([128, KT_TILES, HS], F8, tag="wk", bufs=1)
                wv_sb = p1.tile([128, KT_TILES, HS], F8, tag="wv", bufs=1)
                gw1x_sb = p1.tile([128, KT_TILES, GS], F8, tag="gw1x", bufs=1)

                def load_w(dst, src, ncols):
                    for hh in range(4):
                        nc.sync.dma_start(
                            out=dst[:, hh * 8:(hh + 1) * 8, :],
                            in_=src[hh * 1024:(hh + 1) * 1024, :].rearrange(
                                "(t p) m -> p t m", p=128
                            ),
                        )

                load_w(wq_sb, wq_d, HS)
                xt_tiles = []
                for q in range(IC):
                    xt_q = p1.tile([128, KT_TILES, 512], F8, tag="xt", bufs=2)
                    xt_tiles.append(xt_q)
                isl0 = slice(0, 512)
                for kk in range(4):
                    nc.sync.dma_start(
                        out=xt_tiles[0][:, kk * 8:(kk + 1) * 8, :],
                        in_=xt_f8[kk * 1024:(kk + 1) * 1024, isl0].rearrange(
                            "(t p) i -> p t i", p=128
                        ),
                    )
                load_w(wk_sb, wk_d, HS)
                load_w(wv_sb, wv_d, HS)
                load_w(gw1x_sb, gw1x_d, GS)
                nc.sync.dma_start(out=maskb_sb, in_=maskb_d)
                nc.sync.dma_start(out=diagm_sb, in_=diagm_d)
                nc.sync.dma_start(out=gb1_sb, in_=gb1_d)
                nc.sync.dma_start(out=gb2_sb, in_=gb2_d)

                for q in range(IC):  # 4 quarters of 512 batch elems
                    isl = slice(q * 512, (q + 1) * 512)
                    xt_q = xt_tiles[q]
                    if q + 1 < IC:
                        nxt = slice((q + 1) * 512, (q + 2) * 512)
                        for kk in range(4):
                            nc.sync.dma_start(
                                out=xt_tiles[q + 1][:, kk * 8:(kk + 1) * 8, :],
                                in_=xt_f8[kk * 1024:(kk + 1) * 1024,
                                          nxt].rearrange(
                                    "(t p) i -> p t i", p=128
                                ),
                            )

                    for wsb, dst in ((wq_sb, qt_sb), (wk_sb, kt_sb)):
                        for m in range(4):
                            ps = psum.tile([128, 512], F32, tag="mm", bufs=3,
                                           name="ps_pr")
                            for k in range(KT_TILES // 2):
                                nc.tensor.matmul(
                                    ps,
                                    lhsT=wsb[:, 2 * k:2 * k + 2,
                                             m * 128:(m + 1) * 128],
                                    rhs=xt_q[:, 2 * k:2 * k + 2, :],
                                    start=(k == 0),
                                    stop=(k == KT_TILES // 2 - 1),
                                    perf_mode=DR,
                                )
                            nc.vector.tensor_copy(dst[:, m, isl], ps)
                    # V in natural [j, d] layout: lhsT = X^T tiles
                    for it in range(4):  # 4 i-tiles of 128 in this quarter
                        ps = psum.tile([128, 512], F32, tag="mm", bufs=3,
                                       name="ps_v")
                        for k in range(KT_TILES // 2):
                            nc.tensor.matmul(
                                ps,
                                lhsT=xt_q[:, 2 * k:2 * k + 2,
                                          it * 128:(it + 1) * 128],
                                rhs=wv_sb[:, 2 * k:2 * k + 2, :],
                                start=(k == 0),
                                stop=(k == KT_TILES // 2 - 1),
                                perf_mode=DR,
                            )
                        nc.vector.tensor_scalar_mul(
                            v_sb[:, q * 4 + it, :], ps, 1.0 / W_SCALE
                        )
                    # gate W1 X-part (gh-shard output)
                    ps = psum.tile([128, 512], F32, tag="mm", bufs=3,
                                   name="ps_g1x")
                    for k in range(KT_TILES // 2):
                        nc.tensor.matmul(
                            ps,
                            lhsT=gw1x_sb[:, 2 * k:2 * k + 2, :],
                            rhs=xt_q[:, 2 * k:2 * k + 2, :],
                            start=(k == 0),
                            stop=(k == KT_TILES // 2 - 1),
                            perf_mode=DR,
                        )
                    nc.vector.tensor_scalar_mul(
                        g1x_sb[:, isl], ps, 1.0 / W_SCALE
                    )

            # late warmup: input DMA depends on phase-1 output, so this
            # executes on the CC right before the first real AllGather
            nc.gpsimd.dma_start(out=warm3_i, in_=g1x_sb[:, 0:64])
            nc.gpsimd.collective_compute(
                "AllGather", mybir.AluOpType.bypass, replica_groups=groups,
                ins=[warm3_i.opt()], outs=[warm3_o.opt()],
            )

            # =====================================================
            # Phases 2-4 merged: attention blocks, out_proj chunks and
            # the gate-MLP chain interleaved so collectives overlap PE
            # work. Second-half head order h3..h0 matches the order in
            # which out_proj consumes the hf=1 AllGathers.
            # =====================================================
            with (
                tc.tile_pool(name="p2", bufs=1) as p2,
                tc.tile_pool(name="p34", bufs=1) as p34,
            ):
                # phase-3/4 weights load up front, hidden under phase 2
                cacc = p34.tile([128, 4, B], BF16, tag="cacc", bufs=1)
                wo_sb = p34.tile([128, KT_TILES, HS], F8, tag="wo", bufs=1)
                nc.sync.dma_start(
                    out=wo_sb, in_=wo_d.rearrange("(t p) m -> p t m", p=128)
                )
                gw1c_sb = p34.tile([128, 4, GH], F8, tag="gw1c", bufs=1)
                nc.sync.dma_start(
                    out=gw1c_sb, in_=gw1c_d.rearrange("(t p) m -> p t m", p=128)
                )
                gw2_sb = p34.tile([128, NC_, HS], F8, tag="gw2", bufs=1)
                nc.sync.dma_start(
                    out=gw2_sb, in_=gw2_d.rearrange("(t p) m -> p t m", p=128)
                )
                g1c_sb = p34.tile([128, B], BF16, tag="g1c", bufs=1)
                g1c_sb8 = p34.tile([128, B], F8, tag="g1c8", bufs=1)
                gtf_tiles = {}

                def attn_block(h, q):
                    qsl = slice(q * 512, (q + 1) * 512)
                    den_ps = psum.tile([128, 512], F32, tag="den", bufs=2)
                    ot_ps = psum.tile([128, 512], F32, tag="ot", bufs=2)
                    pt = p2.tile([128, JT, 512], F8E5, tag="pt", bufs=2)
                    for j in range(JT):
                        st = psum.tile([128, 512], F32, tag="mm", bufs=3,
                                       name="st")
                        nc.tensor.matmul(
                            st,
                            lhsT=kt_sb[:, h, j * 128:(j + 1) * 128],
                            rhs=qt_sb[:, h, qsl],
                            start=True,
                            stop=True,
                        )
                        # qt/kt are 64x: fold 1/4096 into the exp scale
                        nc.scalar.activation(
                            pt[:, j, :],
                            st,
                            mybir.ActivationFunctionType.Exp,
                            bias=maskb_sb[:, j:j + 1],
                            scale=SCALE / (W_SCALE * W_SCALE),
                        )
                        # zero the self-attention diagonal block
                        if j // 4 == q:
                            c0 = (j % 4) * 128
                            nc.vector.tensor_mul(
                                pt[:, j, c0:c0 + 128],
                                pt[:, j, c0:c0 + 128],
                                diagm_sb,
                            )
                    for jj in range(JT // 2):
                        nc.tensor.matmul(
                            den_ps,
                            lhsT=ones2_sb,
                            rhs=pt[:, 2 * jj:2 * jj + 2, :],
                            start=(jj == 0),
                            stop=(jj == JT // 2 - 1),
                            perf_mode=DR,
                        )
                        nc.tensor.matmul(
                            ot_ps,
                            lhsT=v_sb[:, 2 * jj:2 * jj + 2,
                                      h * 128:(h + 1) * 128],
                            rhs=pt[:, 2 * jj:2 * jj + 2, :],
                            start=(jj == 0),
                            stop=(jj == JT // 2 - 1),
                            perf_mode=DR,
                        )
                    rec = p2.tile([128, 512], F32, tag="rec", bufs=2)
                    nc.vector.reciprocal_approx_fast(out=rec, in_=den_ps)
                    otc = p2.tile([128, 512], F8, tag="otc", bufs=2)
                    nc.vector.tensor_mul(otc, ot_ps, rec)
                    nc.sync.dma_start(
                        out=ag_in[h][q // 2][:, (q % 2) * 512:
                                             (q % 2) * 512 + 512],
                        in_=otc,
                    )

                def ag_ot(h, hf):
                    nc.gpsimd.collective_compute(
                        "AllGather",
                        mybir.AluOpType.bypass,
                        replica_groups=groups,
                        ins=[ag_in[h][hf].opt()],
                        outs=[ag_out[h][hf].opt()],
                    )

                otg_tiles = {}

                def load_otg(t, ic):
                    otg = p34.tile([128, NC_, 512], F8, tag="otg", bufs=6,
                                   name="otg")
                    nc.sync.dma_start(
                        out=otg,
                        in_=ag_out[t][ic // 2][:, (ic % 2) * 512:
                                               (ic % 2) * 512 + 512].rearrange(
                            "(r p) i -> p r i", p=128
                        ),
                    )
                    otg_tiles[(t, ic)] = otg

                def outproj_chunk(ic, t_list, first, last):
                    """One psum accumulation group per m over all t in
                    t_list (16 DR when t_list is all 4 slots). ACT does
                    the psum->cacc copy; DVE only accumulates the late
                    t0 slot of the tail chunks."""
                    csl = slice(ic * 512, (ic + 1) * 512)
                    for m in range(4):
                        ps = psum.tile([128, 512], F32, tag="mm", bufs=3,
                                       name="ps_wo")
                        for ti, t in enumerate(t_list):
                            otg = otg_tiles[(t, ic)]
                            for r in range(NC_ // 2):
                                nc.tensor.matmul(
                                    ps,
                                    lhsT=wo_sb[:, t * NC_ + 2 * r:
                                               t * NC_ + 2 * r + 2,
                                               m * 128:(m + 1) * 128],
                                    rhs=otg[:, 2 * r:2 * r + 2, :],
                                    start=(ti == 0 and r == 0),
                                    stop=(ti == len(t_list) - 1
                                          and r == NC_ // 2 - 1),
                                    perf_mode=DR,
                                )
                        if first:
                            nc.vector.tensor_scalar_mul(
                                cacc[:, m, csl], ps, 1.0 / W_SCALE
                            )
                        else:
                            nc.vector.scalar_tensor_tensor(
                                cacc[:, m, csl], ps, 1.0 / W_SCALE,
                                cacc[:, m, csl],
                                op0=mybir.AluOpType.mult,
                                op1=mybir.AluOpType.add,
                            )

                def gate_g1c(ic):
                    csl = slice(ic * 512, (ic + 1) * 512)
                    # fp8 copy of this cross^T chunk for the DR g1C matmul
                    cf8 = p34.tile([128, 4, 512], F8, tag="cf8", bufs=2)
                    for m in range(4):
                        nc.vector.tensor_copy(cf8[:, m, :], cacc[:, m, csl])
                    rdt = F8
                    if ic < 2:
                        rs_dst_t = rs_in_c[ic]
                        rs_col = slice(0, 512)
                    else:
                        rs_dst_t = rs_in_t
                        rs_col = slice((ic - 2) * 512, (ic - 2) * 512 + 512)
                    for gm in range(NC_):  # 8 gh-tiles of g1C partial
                        ps = psum.tile([128, 512], F32, tag="mm", bufs=3,
                                       name="ps_g1c")
                        for r in range(2):
                            nc.tensor.matmul(
                                ps,
                                lhsT=gw1c_sb[:, 2 * r:2 * r + 2,
                                             gm * 128:(gm + 1) * 128],
                                rhs=cf8[:, 2 * r:2 * r + 2, :],
                                start=(r == 0),
                                stop=(r == 1),
                                perf_mode=DR,
                            )
                        g1c_ch = p34.tile([128, 512], rdt, tag="g1cch",
                                          bufs=4, name=f"g1cch{ic % 2}")
                        nc.vector.tensor_scalar_mul(g1c_ch, ps, 1.0 / W_SCALE)
                        nc.sync.dma_start(
                            out=rs_dst_t[gm * 128:(gm + 1) * 128, rs_col],
                            in_=g1c_ch,
                        )

                def rs_issue(ic):
                    if ic == 23:
                        i_, o_ = rs_in_t, rs_out_t
                    else:
                        i_, o_ = rs_in_c[ic], rs_out_c[ic]
                    nc.gpsimd.collective_compute(
                        "ReduceScatter",
                        mybir.AluOpType.add,
                        replica_groups=groups,
                        ins=[i_.opt()],
                        outs=[o_.opt()],
                    )

                def pass_b(ic):
                    csl = slice(ic * 512, (ic + 1) * 512)
                    g1c_t = g1c_sb8
                    rs_src = (rs_out_c[ic] if ic < 2 else
                              rs_out_t[:, (ic - 2) * 512:(ic - 2) * 512 + 512])
                    nc.gpsimd.dma_start(out=g1c_t[:, csl], in_=rs_src)
                    gsum = p34.tile([128, 512], F32, tag="gsum", bufs=2)
                    nc.gpsimd.tensor_add(gsum, g1x_sb[:, csl], g1c_t[:, csl])
                    gt_ch = p34.tile([128, 512], F8, tag="gt", bufs=2)
                    nc.scalar.activation(gt_ch, gsum, GELU_FUNC,
                                         bias=gb1_sb, scale=1.0)
                    if ic < 2:
                        nc.gpsimd.dma_start(
                            out=ag2_in_p[:, ic * 512:(ic + 1) * 512],
                            in_=gt_ch)
                        if ic == 0:
                            return
                        i_, o_ = ag2_in_p, ag2_out_p
                    else:
                        nc.gpsimd.dma_start(
                            out=ag2_in_t[:, (ic - 2) * 512:(ic - 2) * 512 + 512],
                            in_=gt_ch)
                        if ic == 2:
                            return
                        i_, o_ = ag2_in_t, ag2_out_t
                    nc.gpsimd.collective_compute(
                        "AllGather",
                        mybir.AluOpType.bypass,
                        replica_groups=groups,
                        ins=[i_.opt()],
                        outs=[o_.opt()],
                    )

                def load_gtf(ic):
                    gtf = p34.tile([128, NC_, 512], F8, tag="gtf", bufs=4,
                                   name=f"gtf{ic}")
                    if ic < 2:
                        src_ = ag2_out_p[:, ic * 512:(ic + 1) * 512]
                    else:
                        src_ = ag2_out_t[:, (ic - 2) * 512:(ic - 2) * 512 + 512]
                    nc.scalar.dma_start(
                        out=gtf,
                        in_=src_.rearrange("(r p) i -> p r i", p=128),
                    )
                    gtf_tiles[ic] = gtf

                def gw2_chunk(ic):
                    csl = slice(ic * 512, (ic + 1) * 512)
                    gtf = gtf_tiles[ic]
                    for m in range(4):
                        ps = psum.tile([128, 512], F32, tag="mm", bufs=3,
                                       name="ps_gw2")
                        for r in range(NC_ // 2):
                            nc.tensor.matmul(
                                ps,
                                lhsT=gw2_sb[:, 2 * r:2 * r + 2,
                                            m * 128:(m + 1) * 128],
                                rhs=gtf[:, 2 * r:2 * r + 2, :],
                                start=(r == 0),
                                stop=(r == NC_ // 2 - 1),
                                perf_mode=DR,
                            )
                        gate_ch = p34.tile([128, 512], BF16, tag="gate",
                                           bufs=2)
                        nc.scalar.activation(
                            gate_ch, ps,
                            mybir.ActivationFunctionType.Sigmoid,
                            bias=gb2_sb[:, m:m + 1], scale=1.0 / W_SCALE,
                        )
                        outt = p34.tile([128, 512], BF16, tag="outt",
                                        bufs=2)
                        nc.vector.tensor_mul(outt, gate_ch, cacc[:, m, csl])
                        nc.sync.dma_start(
                            out=out_d[m * 128:(m + 1) * 128, csl], in_=outt
                        )

                # ---- schedule ----
                # first half: quarters 0,1 head order 3..0 so the hf0
                # AllGathers land in the order out_proj consumes them
                for h in (3, 2, 1, 0):
                    attn_block(h, 0)
                    attn_block(h, 1)
                    ag_ot(h, 0)
                # second half (h3..h0) interleaved with out_proj + gate
                attn_block(3, 2)
                attn_block(3, 3)
                ag_ot(3, 1)
                for t in (3, 2, 1, 0):
                    load_otg(t, 0)
                outproj_chunk(0, (3, 2, 1, 0), first=True, last=True)
                gate_g1c(0)
                rs_issue(0)
                attn_block(2, 2)
                attn_block(2, 3)
                ag_ot(2, 1)
                for t in (3, 2, 1, 0):
                    load_otg(t, 1)
                outproj_chunk(1, (3, 2, 1, 0), first=True, last=True)
                gate_g1c(1)
                rs_issue(1)
                attn_block(1, 2)
                attn_block(1, 3)
                ag_ot(1, 1)
                attn_block(0, 2)
                attn_block(0, 3)
                ag_ot(0, 1)
                pass_b(0)
                pass_b(1)
                load_gtf(0)
                load_gtf(1)
                for t in (3, 2, 1):
                    load_otg(t, 2)
                    load_otg(t, 3)
                outproj_chunk(2, (3, 2, 1), first=True, last=False)
                outproj_chunk(3, (3, 2, 1), first=True, last=False)
                load_otg(0, 2)
                load_otg(0, 3)
                outproj_chunk(2, (0,), first=False, last=True)
                gate_g1c(2)
                outproj_chunk(3, (0,), first=False, last=True)
                gate_g1c(3)
                rs_issue(23)
                gw2_chunk(0)
                gw2_chunk(1)
                pass_b(2)
                load_gtf(2)
                pass_b(3)
                load_gtf(3)
                gw2_chunk(2)
                gw2_chunk(3)

    nc.compile()
    return nc


def _make_in_maps(inputs):
    f32 = np.float32
    f8 = ml_dtypes.float8_e4m3
    f8e5 = ml_dtypes.float8_e5m2
    X = np.asarray(inputs["hidden_states"], dtype=f32)
    mask = np.asarray(inputs["attention_mask"])
    Wq = np.asarray(inputs["Wq"], dtype=f32)
    Wk = np.asarray(inputs["Wk"], dtype=f32)
    Wv = np.asarray(inputs["Wv"], dtype=f32)
    Wo = np.asarray(inputs["Wo"], dtype=f32)
    gW1 = np.asarray(inputs["gW1"], dtype=f32)
    gb1 = np.asarray(inputs["gb1"], dtype=f32)
    gW2 = np.asarray(inputs["gW2"], dtype=f32)
    gb2 = np.asarray(inputs["gb2"], dtype=f32)

    XT = np.ascontiguousarray(X.T)                       # [4096, 2048]
    XT_f8 = XT.astype(f8)
    # Wo row permutation to match per-head AllGather chunk assembly:
    # OT_full row (t*1024 + r*128 + d) holds global head (4r+t), dim d.
    perm = np.empty(HID, dtype=np.int64)
    for t in range(HPC):
        for r in range(NC_):
            g = 4 * r + t
            perm[t * 1024 + r * 128:t * 1024 + (r + 1) * 128] = np.arange(
                g * 128, (g + 1) * 128
            )
    Wo_p = Wo[perm]
    # bias: -EXP_SHIFT for valid keys (prescales exp by 2^-4, cancels in
    # normalization), -1e30 for masked keys
    maskb = np.where(mask, -EXP_SHIFT, -1e30).astype(f32)    # [2048]
    maskb_t = np.ascontiguousarray(maskb.reshape(JT, 128).T)  # [128, 16]
    diagm = (1.0 - np.eye(128, dtype=f32)).astype(f8e5)

    in_maps = []
    for c in range(NC_):
        hsl = slice(c * HS, (c + 1) * HS)
        gsl = slice(c * GS, (c + 1) * GS)
        in_maps.append({
            "xt_f8": XT_f8,
            "wq": np.ascontiguousarray((Wq[:, hsl] * W_SCALE).astype(f8)),
            "wk": np.ascontiguousarray((Wk[:, hsl] * W_SCALE).astype(f8)),
            "wv": np.ascontiguousarray((Wv[:, hsl] * W_SCALE).astype(f8)),
            "wo": np.ascontiguousarray((Wo_p[:, hsl] * W_SCALE).astype(f8)),
            "gw1x": np.ascontiguousarray(
                (gW1[:HID, gsl] * W_SCALE).astype(f8)),
            "gw1c": np.ascontiguousarray(
                (gW1[HID + c * HS:HID + (c + 1) * HS] * W_SCALE).astype(f8)),
            "gw2": np.ascontiguousarray((gW2[:, hsl] * W_SCALE).astype(f8)),
            "gb1": np.ascontiguousarray(gb1[gsl].reshape(GS, 1)),
            "gb2": np.ascontiguousarray(gb2[hsl].reshape(4, 128).T),
            "maskb": maskb_t,
            "diagm": diagm,
        })
    return in_maps


_NC_CACHE = None


def _run(inputs, trace=False):
    global _NC_CACHE
    if _NC_CACHE is None:
        _NC_CACHE = _build_program()
    nc = _NC_CACHE
    in_maps = _make_in_maps(inputs)
    res = bass_utils.run_bass_kernel_spmd(
        nc, in_maps, core_ids=list(range(NC_)), trace=trace
    )
    shards = [np.asarray(res.results[c]["out"]).astype(np.float32)
              for c in range(NC_)]
    gated = np.concatenate(shards, axis=0).T  # gate * cross, [2048, 4096]
    out = np.asarray(inputs["hidden_states"], dtype=np.float32) + gated
    return np.ascontiguousarray(out), res


def kernel(**inputs) -> np.ndarray:
    out, _ = _run(inputs, trace=False)
    return out
